# revision 4
# baseline (speedup 1.0000x reference)
"""Trainium2 Bass kernel for nn_GAttn_67147518705771.

Computes: score = w0*RBF(gf0, s0) + w1*RBF(gf1, s1)  (N x N)
          attn  = score / (rowsum(score) + 0.01)
          out   = attn @ V + V

Sharding: row-parallel over 8 NeuronCores — core c computes output rows
[c*1024, (c+1)*1024); the key/value side (all N=8192 nodes) is replicated.

Per-core algorithm (all on-chip, no N^2 HBM traffic):
  The exp argument E_m[j,i] = -d2_m[j,i]/(2*sigma_m^2) + ln(w_m) is produced
  directly by ONE bf16 matmul per modality with K=24 augmented feature rows
  (3-way bf16 hi/mid/lo splits of the cross/sq terms give ~fp32 accuracy while
  streaming at bf16 speed, 1 col/cycle). ScalarE exponentiates PSUM->SBUF
  (fp16), DVE adds the two modalities, and PE accumulates S @ [V | 1] into
  persistent PSUM, which yields the row sums (ones column) for free. The
  division by (rowsum + eps) commutes with @V, so it is applied to the
  [128,129] accumulators only, then the residual is added and rows stored.
"""

import numpy as np
import ml_dtypes

import concourse.bass as bass
import concourse.tile as tile
import concourse.mybir as mybir
import concourse.bass_utils as bass_utils
from concourse.tile import ScopedClock

BF16 = ml_dtypes.bfloat16
EPS = 0.01
N = 8192          # total nodes (j / key dim)
DG = 3            # geometric feature dim
DV = 128          # value dim
NCORES = 8
NI = N // NCORES  # query rows per core (1024)
KF = 24           # feature rows per modality
CHUNK = 512       # i-chunk per pass (2 passes per core)
NJB = N // 128    # 64 j-blocks


def _split_sync_waits(nc, maxw=1):
    """The walrus build in this environment rejects instructions carrying
    more than one sync wait ("Too many sync wait commands"). Hoist excess
    waits onto single-wait InstNoOp carriers inserted just before the owning
    instruction (same engine => same sequencer stream, so ordering-equivalent).
    Returns (n_insts_split, n_carriers)."""
    n_split = n_carriers = 0
    for f in nc.m.functions:
        for bb in f.blocks:
            insts = list(bb.instructions)
            out, changed = [], False
            for inst in insts:
                si = inst.sync_info
                waits = list(si.on_wait) if si and si.on_wait else []
                if len(waits) > maxw:
                    n_split += 1
                    changed = True
                    for w in waits[:-maxw]:
                        nop = mybir.InstNoOp(name=f"waitnop-{n_carriers}", ins=[], outs=[])
                        n_carriers += 1
                        nop.engine = inst.engine
                        nop.sync_info = mybir.SyncInfo(on_wait=[w], on_update=[])
                        out.append(nop)
                    inst.sync_info = mybir.SyncInfo(
                        on_wait=waits[-maxw:], on_update=list(si.on_update or [])
                    )
                out.append(inst)
            if changed:
                bb.instructions = out
    return n_split, n_carriers


def build_nc(n_j=N, n_i=NI, e_bufs=3):
    """Build the per-core Bass program (SPMD: same program, per-core data)."""
    f32 = mybir.dt.float32
    f16 = mybir.dt.float16
    bf16 = mybir.dt.bfloat16
    njb = n_j // 128
    nchunks = n_i // CHUNK

    nc = bass.Bass("TRN2", target_bir_lowering=False, debug=False)
    L0 = nc.dram_tensor("L0", [KF, n_j], bf16, kind="ExternalInput").ap()
    L1 = nc.dram_tensor("L1", [KF, n_j], bf16, kind="ExternalInput").ap()
    R0 = nc.dram_tensor("R0", [KF, n_i], bf16, kind="ExternalInput").ap()
    R1 = nc.dram_tensor("R1", [KF, n_i], bf16, kind="ExternalInput").ap()
    # V_aug pre-rearranged on host: [128, njb*129] fp16, block jb holds rows
    # jb*128..jb*128+127 of [V | 1].
    VA = nc.dram_tensor("VA", [128, njb * 129], f16, kind="ExternalInput").ap()
    # V residual rows for this core, isub-major: [128, (n_i/128)*128] f32.
    VR = nc.dram_tensor("VR", [128, n_i], f32, kind="ExternalInput").ap()
    OUT = nc.dram_tensor("out", [n_i, DV], f32, kind="ExternalOutput").ap()

    with tile.TileContext(nc) as tc:
        with (
            tc.tile_pool(name="resident", bufs=1) as rpool,
            tc.tile_pool(name="epool", bufs=e_bufs, space="PSUM") as epool,
            tc.tile_pool(name="upool", bufs=1, space="PSUM") as upool,
            tc.tile_pool(name="spool", bufs=3) as spool,
            tc.tile_pool(name="sumpool", bufs=3) as sumpool,
            tc.tile_pool(name="opool", bufs=2) as opool,
            tc.tile_pool(name="scalars", bufs=4) as scpool,
        ):
            l0_sb = rpool.tile([KF, n_j], bf16)
            nc.sync.dma_start(l0_sb[:], L0[:])
            l1_sb = rpool.tile([KF, n_j], bf16)
            nc.sync.dma_start(l1_sb[:], L1[:])
            r0_sb = rpool.tile([KF, n_i], bf16)
            nc.sync.dma_start(r0_sb[:], R0[:])
            r1_sb = rpool.tile([KF, n_i], bf16)
            nc.sync.dma_start(r1_sb[:], R1[:])
            va_sb = rpool.tile([128, njb * 129], f16)
            vstep = max(1, njb // 8)
            for k in range(0, njb, vstep):
                k2 = min(njb, k + vstep)
                nc.sync.dma_start(va_sb[:, k * 129:k2 * 129], VA[:, k * 129:k2 * 129])
            vr_sb = rpool.tile([128, n_i], f32)
            nc.sync.dma_start(vr_sb[:], VR[:])

            for chunk in range(nchunks):
                c0 = chunk * CHUNK
                # U accumulators: 4 x [128, 129]; 3 packed into one bank + 1.
                ua = upool.tile([128, 3 * 129], f32, tag="ua")
                ub = upool.tile([128, 129], f32, tag="ub")

                def usl(isub):
                    return ua[:, isub * 129:(isub + 1) * 129] if isub < 3 else ub[:]

                for jb in range(njb):
                    e = epool.tile([128, 1024], f32, tag="e")
                    nc.tensor.matmul(
                        e[:, 0:512],
                        lhsT=l0_sb[:, jb * 128:(jb + 1) * 128],
                        rhs=r0_sb[:, c0:c0 + CHUNK],
                        start=True, stop=True,
                    )
                    nc.tensor.matmul(
                        e[:, 512:1024],
                        lhsT=l1_sb[:, jb * 128:(jb + 1) * 128],
                        rhs=r1_sb[:, c0:c0 + CHUNK],
                        start=True, stop=True,
                    )
                    s = spool.tile([128, 1024], f16, tag="s")
                    nc.scalar.activation(s[:], e[:], mybir.ActivationFunctionType.Exp)
                    ss = sumpool.tile([128, 512], f16, tag="ss")
                    nc.vector.tensor_add(ss[:], s[:, 0:512], s[:, 512:1024])
                    for isub in range(4):
                        # start=True clears has_written for the WHOLE bank, so
                        # only the first matmul touching each bank (isub 0 for
                        # ua, isub 3 for ub) may set it; the other subblocks'
                        # first write lands on cleared bits (= overwrite) and
                        # subsequent j-blocks accumulate.
                        nc.tensor.matmul(
                            usl(isub),
                            lhsT=ss[:, isub * 128:(isub + 1) * 128],
                            rhs=va_sb[:, jb * 129:(jb + 1) * 129],
                            start=(jb == 0 and isub in (0, 3)),
                            stop=(jb == njb - 1),
                            skip_group_check=True,
                        )

                for isub in range(4):
                    g = chunk * 4 + isub
                    ut = usl(isub)
                    rt = scpool.tile([128, 1], f32, tag="rt")
                    nc.vector.tensor_scalar_add(rt[:], ut[:, 128:129], EPS)
                    ri = scpool.tile([128, 1], f32, tag="ri")
                    nc.vector.reciprocal(ri[:], rt[:])
                    ot = opool.tile([128, DV], f32, tag="ot")
                    nc.vector.tensor_scalar_mul(ot[:], ut[:, 0:DV], ri[:])
                    nc.vector.tensor_add(ot[:], ot[:], vr_sb[:, g * 128:(g + 1) * 128])
                    nc.sync.dma_start(OUT[g * 128:(g + 1) * 128, :], ot[:])

    ns, ncar = _split_sync_waits(nc)
    print(f"[kernel] split {ns} insts, {ncar} wait carriers", flush=True)
    return nc


def _split3(v):
    v1 = v.astype(BF16).astype(np.float32)
    v2 = (v - v1).astype(BF16).astype(np.float32)
    v3 = (v - v1 - v2).astype(BF16).astype(np.float32)
    return v1, v2, v3


def _build_features(gf, sigma, w):
    """L [KF, N] (j-side) and R [KF, N] (i-side) bf16 feature rows such that
    (L.T @ R)[j, i] = -d2[j,i]/(2 sigma^2) + ln(w) to ~1e-5."""
    gf = np.asarray(gf, dtype=np.float32)
    n = gf.shape[0]
    g = np.float32(1.0 / (2.0 * sigma * sigma))
    sq = (gf * gf).sum(axis=1)
    a = 2.0 * g * gf            # j-side cross
    b = gf                      # i-side cross
    dterm = -g * sq             # j-side
    c = -g * sq + np.float32(np.log(w))  # i-side

    a1, a2, a3 = _split3(a)
    b1, b2, b3 = _split3(b)
    d1, d2_, d3 = _split3(dterm)
    c1, c2, c3 = _split3(c)
    ones = np.ones(n, np.float32)

    Lrows, Rrows = [], []
    for ap, bp in [(a1, b1), (a1, b2), (a2, b1), (a2, b2), (a3, b1), (a1, b3)]:
        for d in range(DG):
            Lrows.append(ap[:, d])
            Rrows.append(bp[:, d])
    for dd in (d1, d2_, d3):
        Lrows.append(dd)
        Rrows.append(ones)
    for cc in (c1, c2, c3):
        Lrows.append(ones)
        Rrows.append(cc)
    L = np.stack(Lrows).astype(BF16)
    R = np.stack(Rrows).astype(BF16)
    return L, R


def _prepare_inputs(gf0, gf1, node_v_feats, weights, sigmas, n_cores=NCORES):
    """Host-side preprocessing -> per-core in_maps."""
    weights = np.asarray(weights, np.float32)
    sigmas = np.asarray(sigmas, np.float32)
    V = np.asarray(node_v_feats, np.float32)
    n = V.shape[0]
    ni = n // n_cores
    njb = n // 128

    L0, R0full = _build_features(gf0, float(sigmas[0]), float(weights[0]))
    L1, R1full = _build_features(gf1, float(sigmas[1]), float(weights[1]))

    vaug = np.concatenate([V, np.ones((n, 1), np.float32)], axis=1)  # [n, 129]
    # [n, 129] -> [njb, 128, 129] -> [128, njb*129]
    va = np.ascontiguousarray(
        vaug.reshape(njb, 128, 129).transpose(1, 0, 2).reshape(128, njb * 129)
    ).astype(np.float16)

    in_maps = []
    for c in range(n_cores):
        rows = slice(c * ni, (c + 1) * ni)
        vr = np.ascontiguousarray(
            V[rows].reshape(ni // 128, 128, DV).transpose(1, 0, 2).reshape(128, ni)
        )
        in_maps.append({
            "L0": np.ascontiguousarray(L0),
            "L1": np.ascontiguousarray(L1),
            "R0": np.ascontiguousarray(R0full[:, rows]),
            "R1": np.ascontiguousarray(R1full[:, rows]),
            "VA": va,
            "VR": vr,
        })
    return in_maps


_NC_CACHE = {}


def _get_nc(n_j=N, n_i=NI):
    key = (n_j, n_i)
    if key not in _NC_CACHE:
        _NC_CACHE[key] = build_nc(n_j, n_i)
    return _NC_CACHE[key]


def kernel(gf0, gf1, node_v_feats, weights, sigmas):
    in_maps = _prepare_inputs(gf0, gf1, node_v_feats, weights, sigmas)
    nc = _get_nc()
    res = bass_utils.run_bass_kernel_spmd(nc, in_maps, core_ids=list(range(NCORES)))
    out = np.concatenate([res.results[c]["out"] for c in range(NCORES)], axis=0)
    return np.ascontiguousarray(out.astype(np.float32))


# revision 6
# speedup vs baseline: 1.0018x; 1.0018x over previous
"""Trainium2 Bass kernel for nn_GAttn_67147518705771.

Computes: score = w0*RBF(gf0, s0) + w1*RBF(gf1, s1)  (N x N)
          attn  = score / (rowsum(score) + 0.01)
          out   = attn @ V + V

Sharding: row-parallel over 8 NeuronCores — core c computes output rows
[c*1024, (c+1)*1024); the key/value side (all N=8192 nodes) is replicated.

Per-core algorithm (all on-chip, no N^2 HBM traffic):
  The exp argument E_m[j,i] = -d2_m[j,i]/(2*sigma_m^2) + ln(w_m) is produced
  directly by ONE bf16 matmul per modality with K=24 augmented feature rows
  (3-way bf16 hi/mid/lo splits of the cross/sq terms give ~fp32 accuracy while
  streaming at bf16 speed, 1 col/cycle). ScalarE exponentiates PSUM->SBUF
  (fp16) in 1536-element ops spanning 3 j-blocks x 2 modalities, DVE adds the
  two modalities, and PE accumulates S @ [V | 1] into persistent PSUM, which
  yields the row sums (ones column) for free. The division by (rowsum + eps)
  commutes with @V, so it is applied to the [128,129] accumulators only, then
  the residual is added and rows stored.

PSUM (8 banks): E tiles [128,1536] (3 banks) double-buffered = 6, U
accumulator [128,258] (1 bank) double-buffered across i-chunks = 2.
start=True clears has_written for the WHOLE bank, so only the first matmul
per accumulation bank sets it.
"""

import numpy as np
import ml_dtypes

import concourse.bass as bass
import concourse.tile as tile
import concourse.mybir as mybir
import concourse.bass_utils as bass_utils

BF16 = ml_dtypes.bfloat16
EPS = 0.01
N = 8192          # total nodes (j / key dim)
DG = 3            # geometric feature dim
DV = 128          # value dim
NCORES = 8
NI = N // NCORES  # query rows per core (1024)
KF = 24           # feature rows per modality
CHUNK = 256       # i-chunk per pass (4 passes per core)
GRP = 3           # j-blocks per E tile / ACT op


def _split_sync_waits(nc, maxw=1):
    """The walrus build in this environment rejects instructions carrying
    more than one sync wait ("Too many sync wait commands"). Hoist excess
    waits onto single-wait InstNoOp carriers inserted just before the owning
    instruction (same engine => same sequencer stream, so ordering-equivalent).
    Returns (n_insts_split, n_carriers)."""
    n_split = n_carriers = 0
    for f in nc.m.functions:
        for bb in f.blocks:
            insts = list(bb.instructions)
            out, changed = [], False
            for inst in insts:
                si = inst.sync_info
                waits = list(si.on_wait) if si and si.on_wait else []
                if len(waits) > maxw:
                    n_split += 1
                    changed = True
                    for w in waits[:-maxw]:
                        nop = mybir.InstNoOp(name=f"waitnop-{n_carriers}", ins=[], outs=[])
                        n_carriers += 1
                        nop.engine = inst.engine
                        nop.sync_info = mybir.SyncInfo(on_wait=[w], on_update=[])
                        out.append(nop)
                    inst.sync_info = mybir.SyncInfo(
                        on_wait=waits[-maxw:], on_update=list(si.on_update or [])
                    )
                out.append(inst)
            if changed:
                bb.instructions = out
    return n_split, n_carriers


def build_nc(n_j=N, n_i=NI):
    """Build the per-core Bass program (SPMD: same program, per-core data)."""
    f32 = mybir.dt.float32
    f16 = mybir.dt.float16
    bf16 = mybir.dt.bfloat16
    njb = n_j // 128
    nchunks = n_i // CHUNK
    # resident input tiles are split into <=8 pieces for fine-grained
    # DMA->compute overlap at startup
    step = (njb + 7) // 8

    nc = bass.Bass("TRN2", target_bir_lowering=False, debug=False)
    L0 = nc.dram_tensor("L0", [KF, n_j], bf16, kind="ExternalInput").ap()
    L1 = nc.dram_tensor("L1", [KF, n_j], bf16, kind="ExternalInput").ap()
    R0 = nc.dram_tensor("R0", [KF, n_i], bf16, kind="ExternalInput").ap()
    R1 = nc.dram_tensor("R1", [KF, n_i], bf16, kind="ExternalInput").ap()
    # V_aug pre-rearranged on host: [128, njb*129] fp16, block jb holds rows
    # jb*128..jb*128+127 of [V | 1].
    VA = nc.dram_tensor("VA", [128, njb * 129], f16, kind="ExternalInput").ap()
    # V residual rows for this core, 128-row-block-major: [128, n_i] f32.
    VR = nc.dram_tensor("VR", [128, n_i], f32, kind="ExternalInput").ap()
    OUT = nc.dram_tensor("out", [n_i, DV], f32, kind="ExternalOutput").ap()

    groups = [list(range(g, min(g + GRP, njb))) for g in range(0, njb, GRP)]

    with tile.TileContext(nc) as tc:
        with (
            tc.tile_pool(name="resident", bufs=1) as rpool,
            tc.tile_pool(name="epool", bufs=2, space="PSUM") as epool,
            tc.tile_pool(name="upool", bufs=2, space="PSUM") as upool,
            tc.tile_pool(name="spool", bufs=3) as spool,
            tc.tile_pool(name="sumpool", bufs=3) as sumpool,
            tc.tile_pool(name="opool", bufs=2) as opool,
            tc.tile_pool(name="scalars", bufs=4) as scpool,
        ):
            # Dummy exp to trigger the ACT exp-table load while input DMAs run.
            dummy = scpool.tile([128, 1], f32, tag="dummy")
            nc.gpsimd.memset(dummy[:], 0.0)
            dummy2 = scpool.tile([128, 1], f32, tag="dummy2")
            nc.scalar.activation(dummy2[:], dummy[:], mybir.ActivationFunctionType.Exp)

            r0_sb = rpool.tile([KF, n_i], bf16)
            nc.sync.dma_start(r0_sb[:], R0[:])
            r1_sb = rpool.tile([KF, n_i], bf16)
            nc.sync.dma_start(r1_sb[:], R1[:])

            l_tiles = []  # [modality][piece] -> tile of [KF, step*128]
            for m, Lap in ((0, L0), (1, L1)):
                tiles = []
                for p in range(0, njb, step):
                    p2 = min(njb, p + step)
                    t = rpool.tile([KF, (p2 - p) * 128], bf16, name=f"l{m}_{p}")
                    nc.sync.dma_start(t[:], Lap[:, p * 128:p2 * 128])
                    tiles.append(t)
                l_tiles.append(tiles)

            va_tiles = []
            for p in range(0, njb, step):
                p2 = min(njb, p + step)
                t = rpool.tile([128, (p2 - p) * 129], f16, name=f"va_{p}")
                nc.sync.dma_start(t[:], VA[:, p * 129:p2 * 129])
                va_tiles.append(t)

            vr_sb = rpool.tile([128, n_i], f32)
            nc.sync.dma_start(vr_sb[:], VR[:])

            def lsl(m, jb):  # lhsT feature slice [KF, 128] for modality m
                t = l_tiles[m][jb // step]
                o = (jb % step) * 128
                return t[:, o:o + 128]

            def vasl(jb):  # V_aug block [128, 129]
                t = va_tiles[jb // step]
                o = (jb % step) * 129
                return t[:, o:o + 129]

            for chunk in range(nchunks):
                c0 = chunk * CHUNK
                # U accumulator: 2 subblocks x 129 cols packed in ONE bank.
                u = upool.tile([128, 2 * 129], f32, tag="u")

                for jbs in groups:
                    w = len(jbs) * 512
                    e = epool.tile([128, GRP * 512], f32, tag="e")
                    for t, jb in enumerate(jbs):
                        nc.tensor.matmul(
                            e[:, t * 512:t * 512 + 256],
                            lhsT=lsl(0, jb),
                            rhs=r0_sb[:, c0:c0 + CHUNK],
                            start=True, stop=True,
                        )
                        nc.tensor.matmul(
                            e[:, t * 512 + 256:(t + 1) * 512],
                            lhsT=lsl(1, jb),
                            rhs=r1_sb[:, c0:c0 + CHUNK],
                            start=True, stop=True,
                        )
                    s = spool.tile([128, GRP * 512], f16, tag="s")
                    nc.scalar.activation(
                        s[:, 0:w], e[:, 0:w], mybir.ActivationFunctionType.Exp
                    )
                    ss = sumpool.tile([128, GRP * 256], f16, tag="ss")
                    for t, jb in enumerate(jbs):
                        nc.vector.tensor_add(
                            ss[:, t * 256:(t + 1) * 256],
                            s[:, t * 512:t * 512 + 256],
                            s[:, t * 512 + 256:(t + 1) * 512],
                        )
                    for t, jb in enumerate(jbs):
                        for isub in range(2):
                            # start=True clears has_written for the WHOLE
                            # bank: only the first matmul of the chunk sets
                            # it; isub 1's first write lands on cleared bits
                            # (= overwrite) and later j-blocks accumulate.
                            nc.tensor.matmul(
                                u[:, isub * 129:(isub + 1) * 129],
                                lhsT=ss[:, t * 256 + isub * 128:t * 256 + (isub + 1) * 128],
                                rhs=vasl(jb),
                                start=(jb == 0 and isub == 0),
                                stop=(jb == njb - 1),
                                skip_group_check=True,
                            )

                for isub in range(2):
                    g = chunk * 2 + isub
                    ut = u[:, isub * 129:(isub + 1) * 129]
                    rt = scpool.tile([128, 1], f32, tag="rt")
                    nc.vector.tensor_scalar_add(rt[:], ut[:, 128:129], EPS)
                    ri = scpool.tile([128, 1], f32, tag="ri")
                    nc.vector.reciprocal(ri[:], rt[:])
                    ot = opool.tile([128, DV], f32, tag="ot")
                    nc.vector.tensor_scalar_mul(ot[:], ut[:, 0:DV], ri[:])
                    nc.vector.tensor_add(ot[:], ot[:], vr_sb[:, g * 128:(g + 1) * 128])
                    nc.sync.dma_start(OUT[g * 128:(g + 1) * 128, :], ot[:])

    _split_sync_waits(nc)
    return nc


def _split3(v):
    v1 = v.astype(BF16).astype(np.float32)
    v2 = (v - v1).astype(BF16).astype(np.float32)
    v3 = (v - v1 - v2).astype(BF16).astype(np.float32)
    return v1, v2, v3


def _build_features(gf, sigma, w):
    """L [KF, N] (j-side) and R [KF, N] (i-side) bf16 feature rows such that
    (L.T @ R)[j, i] = -d2[j,i]/(2 sigma^2) + ln(w) to ~1e-5."""
    gf = np.asarray(gf, dtype=np.float32)
    n = gf.shape[0]
    g = np.float32(1.0 / (2.0 * sigma * sigma))
    sq = (gf * gf).sum(axis=1)
    a = 2.0 * g * gf            # j-side cross
    b = gf                      # i-side cross
    dterm = -g * sq             # j-side
    c = -g * sq + np.float32(np.log(w))  # i-side

    a1, a2, a3 = _split3(a)
    b1, b2, b3 = _split3(b)
    d1, d2_, d3 = _split3(dterm)
    c1, c2, c3 = _split3(c)
    ones = np.ones(n, np.float32)

    Lrows, Rrows = [], []
    for ap, bp in [(a1, b1), (a1, b2), (a2, b1), (a2, b2), (a3, b1), (a1, b3)]:
        for d in range(DG):
            Lrows.append(ap[:, d])
            Rrows.append(bp[:, d])
    for dd in (d1, d2_, d3):
        Lrows.append(dd)
        Rrows.append(ones)
    for cc in (c1, c2, c3):
        Lrows.append(ones)
        Rrows.append(cc)
    L = np.stack(Lrows).astype(BF16)
    R = np.stack(Rrows).astype(BF16)
    return L, R


def _prepare_inputs(gf0, gf1, node_v_feats, weights, sigmas, n_cores=NCORES):
    """Host-side preprocessing -> per-core in_maps."""
    weights = np.asarray(weights, np.float32)
    sigmas = np.asarray(sigmas, np.float32)
    V = np.asarray(node_v_feats, np.float32)
    n = V.shape[0]
    ni = n // n_cores
    njb = n // 128

    L0, R0full = _build_features(gf0, float(sigmas[0]), float(weights[0]))
    L1, R1full = _build_features(gf1, float(sigmas[1]), float(weights[1]))

    vaug = np.concatenate([V, np.ones((n, 1), np.float32)], axis=1)  # [n, 129]
    va = np.ascontiguousarray(
        vaug.reshape(njb, 128, 129).transpose(1, 0, 2).reshape(128, njb * 129)
    ).astype(np.float16)

    in_maps = []
    for c in range(n_cores):
        rows = slice(c * ni, (c + 1) * ni)
        vr = np.ascontiguousarray(
            V[rows].reshape(ni // 128, 128, DV).transpose(1, 0, 2).reshape(128, ni)
        )
        in_maps.append({
            "L0": np.ascontiguousarray(L0),
            "L1": np.ascontiguousarray(L1),
            "R0": np.ascontiguousarray(R0full[:, rows]),
            "R1": np.ascontiguousarray(R1full[:, rows]),
            "VA": va,
            "VR": vr,
        })
    return in_maps


_NC_CACHE = {}


def _get_nc(n_j=N, n_i=NI):
    key = (n_j, n_i)
    if key not in _NC_CACHE:
        _NC_CACHE[key] = build_nc(n_j, n_i)
    return _NC_CACHE[key]


_EXEC_CACHE = {}


def _get_executor(nc, n_cores):
    """Cached jitted shard_map executor (avoids re-tracing per call)."""
    key = (id(nc), n_cores)
    if key in _EXEC_CACHE:
        return _EXEC_CACHE[key]
    import jax
    from jax.experimental.shard_map import shard_map
    from jax.sharding import Mesh, PartitionSpec
    from concourse.bass2jax import (
        install_neuronx_cc_hook,
        _bass_exec_p,
        partition_id_tensor,
    )

    install_neuronx_cc_hook()

    partition_name = nc.partition_id_tensor.name if nc.partition_id_tensor else None
    in_names, out_names, out_avals = [], [], []
    for alloc in nc.m.functions[0].allocations:
        if not isinstance(alloc, mybir.MemoryLocationSet):
            continue
        name = alloc.memorylocations[0].name
        if alloc.kind == "ExternalInput":
            if name != partition_name:
                in_names.append(name)
        elif alloc.kind == "ExternalOutput":
            out_names.append(name)
            out_avals.append(
                jax.core.ShapedArray(tuple(alloc.tensor_shape), mybir.dt.np(alloc.dtype))
            )
    n_params = len(in_names)
    all_names = list(in_names) + list(out_names)
    if partition_name is not None:
        all_names.append(partition_name)

    def _body(*args):
        operands = list(args)
        if partition_name is not None:
            operands.append(partition_id_tensor())
        outs = _bass_exec_p.bind(
            *operands,
            out_avals=tuple(out_avals),
            in_names=tuple(all_names),
            out_names=tuple(out_names),
            lowering_input_output_aliases=(),
            sim_require_finite=True,
            sim_require_nnan=True,
            nc=nc,
        )
        return tuple(outs)

    devices = jax.devices()[:n_cores]
    mesh = Mesh(np.asarray(devices), ("core",))
    n_outs = len(out_names)
    sharded = jax.jit(
        shard_map(
            _body,
            mesh=mesh,
            in_specs=(PartitionSpec("core"),) * (n_params + n_outs),
            out_specs=(PartitionSpec("core"),) * n_outs,
            check_rep=False,
        ),
        donate_argnums=tuple(range(n_params, n_params + n_outs)),
        keep_unused=True,
    )
    entry = (sharded, in_names, out_names, out_avals)
    _EXEC_CACHE[key] = entry
    return entry


def _run(nc, in_maps, n_cores):
    sharded, in_names, out_names, out_avals = _get_executor(nc, n_cores)
    concat_in = [
        np.concatenate([in_maps[c][name] for c in range(n_cores)], axis=0)
        for name in in_names
    ]
    concat_zeros = [
        np.zeros((n_cores * a.shape[0], *a.shape[1:]), a.dtype) for a in out_avals
    ]
    out_arrs = sharded(*concat_in, *concat_zeros)
    return [
        {
            name: np.asarray(out_arrs[i]).reshape(n_cores, *out_avals[i].shape)[c]
            for i, name in enumerate(out_names)
        }
        for c in range(n_cores)
    ]


def kernel(gf0, gf1, node_v_feats, weights, sigmas):
    in_maps = _prepare_inputs(gf0, gf1, node_v_feats, weights, sigmas)
    nc = _get_nc()
    results = _run(nc, in_maps, NCORES)
    out = np.concatenate([results[c]["out"] for c in range(NCORES)], axis=0)
    return np.ascontiguousarray(out.astype(np.float32))


# revision 14
# speedup vs baseline: 1.0428x; 1.0410x over previous
"""Trainium2 Bass kernel for nn_GAttn_67147518705771.

Computes: score = w0*RBF(gf0, s0) + w1*RBF(gf1, s1)  (N x N)
          attn  = score / (rowsum(score) + 0.01)
          out   = attn @ V + V

Sharding: row-parallel over 8 NeuronCores — core c computes output rows
[c*1024, (c+1)*1024); the key/value side (all N=8192 nodes) is replicated.

Per-core algorithm (all on-chip, no N^2 HBM traffic):
  The exp argument E_m[j,i] = -d2_m[j,i]/(2*sigma_m^2) + ln(w_m) is produced
  directly by ONE bf16 matmul per modality with K=24 augmented feature rows
  (3-way bf16 hi/mid/lo splits of the cross/sq terms give ~fp32 accuracy while
  streaming at bf16 speed, 1 col/cycle). ScalarE exponentiates PSUM->SBUF
  (fp16) in 1536-element ops spanning 3 j-blocks x 2 modalities, DVE adds the
  two modalities, and PE accumulates S @ [V | 1] into persistent PSUM, which
  yields the row sums (ones column) for free. The division by (rowsum + eps)
  commutes with @V, so it is applied to the [128,129] accumulators only, then
  the residual is added and rows stored.

PSUM (8 banks): E tiles [128,1536] (3 banks) double-buffered = 6, U
accumulator [128,258] (1 bank) double-buffered across i-chunks = 2.
start=True clears has_written for the WHOLE bank, so only the first matmul
per accumulation bank sets it.
"""

import numpy as np
import ml_dtypes

import concourse.bass as bass
import concourse.tile as tile
import concourse.mybir as mybir
import concourse.bass_utils as bass_utils

BF16 = ml_dtypes.bfloat16
EPS = 0.01
N = 8192          # total nodes (j / key dim)
DG = 3            # geometric feature dim
DV = 128          # value dim
NCORES = 8
NI = N // NCORES  # query rows per core (1024)
KF = 24           # feature rows per modality
CHUNK = 256       # i-chunk per pass (4 passes per core)
GRP = 3           # j-blocks per E tile / ACT op


def _split_sync_waits(nc, maxw=1):
    """The walrus build in this environment rejects instructions carrying
    more than one sync wait ("Too many sync wait commands"). Hoist excess
    waits onto single-wait InstNoOp carriers inserted just before the owning
    instruction (same engine => same sequencer stream, so ordering-equivalent).
    Returns (n_insts_split, n_carriers)."""
    n_split = n_carriers = 0
    for f in nc.m.functions:
        for bb in f.blocks:
            insts = list(bb.instructions)
            out, changed = [], False
            for inst in insts:
                si = inst.sync_info
                waits = list(si.on_wait) if si and si.on_wait else []
                if len(waits) > maxw:
                    n_split += 1
                    changed = True
                    for w in waits[:-maxw]:
                        nop = mybir.InstNoOp(name=f"waitnop-{n_carriers}", ins=[], outs=[])
                        n_carriers += 1
                        nop.engine = inst.engine
                        nop.sync_info = mybir.SyncInfo(on_wait=[w], on_update=[])
                        out.append(nop)
                    inst.sync_info = mybir.SyncInfo(
                        on_wait=waits[-maxw:], on_update=list(si.on_update or [])
                    )
                out.append(inst)
            if changed:
                bb.instructions = out
    return n_split, n_carriers


def build_nc(n_j=N, n_i=NI):
    """Build the per-core Bass program (SPMD: same program, per-core data)."""
    f32 = mybir.dt.float32
    f16 = mybir.dt.float16
    bf16 = mybir.dt.bfloat16
    njb = n_j // 128
    nchunks = n_i // CHUNK
    # resident input tiles are split into <=8 pieces for fine-grained
    # DMA->compute overlap at startup
    step = (njb + 7) // 8

    nc = bass.Bass("TRN2", target_bir_lowering=False, debug=False)
    L0 = nc.dram_tensor("L0", [KF, n_j], bf16, kind="ExternalInput").ap()
    L1 = nc.dram_tensor("L1", [KF, n_j], bf16, kind="ExternalInput").ap()
    R0 = nc.dram_tensor("R0", [KF, n_i], bf16, kind="ExternalInput").ap()
    R1 = nc.dram_tensor("R1", [KF, n_i], bf16, kind="ExternalInput").ap()
    # V_aug pre-rearranged on host: [128, njb*129] fp16, block jb holds rows
    # jb*128..jb*128+127 of [V | 1].
    VA = nc.dram_tensor("VA", [128, njb * 129], f16, kind="ExternalInput").ap()
    # V residual rows for this core, 128-row-block-major: [128, n_i] f32.
    VR = nc.dram_tensor("VR", [128, n_i], f32, kind="ExternalInput").ap()
    OUT = nc.dram_tensor("out", [n_i, DV], f32, kind="ExternalOutput").ap()

    groups = [list(range(g, min(g + GRP, njb))) for g in range(0, njb, GRP)]

    with tile.TileContext(nc) as tc:
        with (
            tc.tile_pool(name="resident", bufs=1) as rpool,
            tc.tile_pool(name="epool", bufs=2, space="PSUM") as epool,
            tc.tile_pool(name="upool", bufs=2, space="PSUM") as upool,
            tc.tile_pool(name="spool", bufs=3) as spool,
            tc.tile_pool(name="sumpool", bufs=3) as sumpool,
            tc.tile_pool(name="opool", bufs=4) as opool,
            tc.tile_pool(name="scalars", bufs=4) as scpool,
        ):
            # Spread the first group's operands across independent DMA-issue
            # rails (each rail serializes at ~0.6-1us per dma_start): the
            # critical chain for the first E matmuls is r0+l0p0 (sync rail),
            # r1 (gpsimd) and l1p0 (tensor rail); va0 follows on gpsimd.
            # Later pieces are interleaved on sync in compute order.
            r0_sb = rpool.tile([KF, n_i], bf16)
            nc.sync.dma_start(r0_sb[:], R0[:])
            r1_sb = rpool.tile([KF, n_i], bf16)
            nc.gpsimd.dma_start(r1_sb[:], R1[:])

            l_tiles = [[], []]
            va_tiles = []
            for p in range(0, njb, step):
                p2 = min(njb, p + step)
                for m, Lap in ((0, L0), (1, L1)):
                    eng = nc.sync if p > 0 else (nc.sync if m == 0 else nc.scalar)
                    t = rpool.tile([KF, (p2 - p) * 128], bf16, name=f"l{m}_{p}")
                    eng.dma_start(t[:], Lap[:, p * 128:p2 * 128])
                    l_tiles[m].append(t)
                eng = nc.gpsimd if p == 0 else nc.sync
                t = rpool.tile([128, (p2 - p) * 129], f16, name=f"va_{p}")
                eng.dma_start(t[:], VA[:, p * 129:p2 * 129])
                va_tiles.append(t)

            vr_sb = rpool.tile([128, n_i], f32)
            nc.sync.dma_start(vr_sb[:], VR[:])

            # Dummy exp (after the ACT-rail DMA issue) pre-loads the ACT
            # exp-table while the input DMAs stream in.
            dummy = scpool.tile([128, 1], f32, tag="dummy")
            nc.vector.memset(dummy[:], 0.0)
            dummy2 = scpool.tile([128, 1], f32, tag="dummy2")
            nc.scalar.activation(dummy2[:], dummy[:], mybir.ActivationFunctionType.Exp)

            def lsl(m, jb):  # lhsT feature slice [KF, 128] for modality m
                t = l_tiles[m][jb // step]
                o = (jb % step) * 128
                return t[:, o:o + 128]

            def vasl(jb):  # V_aug block [128, 129]
                t = va_tiles[jb // step]
                o = (jb % step) * 129
                return t[:, o:o + 129]

            # Chunks are processed in interleaved PAIRS: while chunk c0's exp
            # runs on ScalarE, the PE computes chunk c1's E matmuls, so the
            # PE program order never stalls on the last exp of a chunk except
            # at pair boundaries.
            assert nchunks % 2 == 0
            for cpair in range(nchunks // 2):
                chunks = (2 * cpair, 2 * cpair + 1)
                # Per-chunk U accumulator: 2 subblocks x 129 cols in ONE bank.
                u_t = [upool.tile([128, 2 * 129], f32, tag="u", name=f"u_{c}")
                       for c in chunks]

                for jbs in groups:
                    w = len(jbs) * 512
                    for ci, chunk in enumerate(chunks):
                        c0 = chunk * CHUNK
                        u = u_t[ci]
                        e = epool.tile([128, GRP * 512], f32, tag="e",
                                       name=f"e_{chunk}_{jbs[0]}")
                        for t, jb in enumerate(jbs):
                            nc.tensor.matmul(
                                e[:, t * 512:t * 512 + 256],
                                lhsT=lsl(0, jb),
                                rhs=r0_sb[:, c0:c0 + CHUNK],
                                start=True, stop=True,
                            )
                            nc.tensor.matmul(
                                e[:, t * 512 + 256:(t + 1) * 512],
                                lhsT=lsl(1, jb),
                                rhs=r1_sb[:, c0:c0 + CHUNK],
                                start=True, stop=True,
                            )
                        s = spool.tile([128, GRP * 512], f16, tag="s",
                                       name=f"s_{chunk}_{jbs[0]}")
                        nc.scalar.activation(
                            s[:, 0:w], e[:, 0:w], mybir.ActivationFunctionType.Exp
                        )
                        ss = sumpool.tile([128, GRP * 256], f16, tag="ss",
                                          name=f"ss_{chunk}_{jbs[0]}")
                        for t, jb in enumerate(jbs):
                            nc.vector.tensor_add(
                                ss[:, t * 256:(t + 1) * 256],
                                s[:, t * 512:t * 512 + 256],
                                s[:, t * 512 + 256:(t + 1) * 512],
                            )
                        for t, jb in enumerate(jbs):
                            for isub in range(2):
                                # start=True clears has_written for the WHOLE
                                # bank: only the first matmul touching the
                                # bank in this chunk sets it; isub 1's first
                                # write lands on cleared bits (= overwrite)
                                # and later j-blocks accumulate.
                                nc.tensor.matmul(
                                    u[:, isub * 129:(isub + 1) * 129],
                                    lhsT=ss[:, t * 256 + isub * 128:t * 256 + (isub + 1) * 128],
                                    rhs=vasl(jb),
                                    start=(jb == 0 and isub == 0),
                                    stop=(jb == njb - 1),
                                    skip_group_check=True,
                                )

                for ci, chunk in enumerate(chunks):
                    for isub in range(2):
                        g = chunk * 2 + isub
                        ut = u_t[ci][:, isub * 129:(isub + 1) * 129]
                        rt = scpool.tile([128, 1], f32, tag="rt", name=f"rt_{g}")
                        nc.vector.tensor_scalar_add(rt[:], ut[:, 128:129], EPS)
                        ri = scpool.tile([128, 1], f32, tag="ri", name=f"ri_{g}")
                        nc.vector.reciprocal(ri[:], rt[:])
                        ot = opool.tile([128, DV], f32, tag="ot", name=f"ot_{g}")
                        nc.vector.tensor_scalar_mul(ot[:], ut[:, 0:DV], ri[:])
                        nc.vector.tensor_add(ot[:], ot[:], vr_sb[:, g * 128:(g + 1) * 128])
                        out_eng = nc.sync if isub == 0 else nc.gpsimd
                        out_eng.dma_start(OUT[g * 128:(g + 1) * 128, :], ot[:])

    _split_sync_waits(nc)
    return nc


def _split3(v):
    v1 = v.astype(BF16).astype(np.float32)
    v2 = (v - v1).astype(BF16).astype(np.float32)
    v3 = (v - v1 - v2).astype(BF16).astype(np.float32)
    return v1, v2, v3


def _build_features(gf, sigma, w):
    """L [KF, N] (j-side) and R [KF, N] (i-side) bf16 feature rows such that
    (L.T @ R)[j, i] = -d2[j,i]/(2 sigma^2) + ln(w) to ~1e-5."""
    gf = np.asarray(gf, dtype=np.float32)
    n = gf.shape[0]
    g = np.float32(1.0 / (2.0 * sigma * sigma))
    sq = (gf * gf).sum(axis=1)
    a = 2.0 * g * gf            # j-side cross
    b = gf                      # i-side cross
    dterm = -g * sq             # j-side
    c = -g * sq + np.float32(np.log(w))  # i-side

    a1, a2, a3 = _split3(a)
    b1, b2, b3 = _split3(b)
    d1, d2_, d3 = _split3(dterm)
    c1, c2, c3 = _split3(c)
    ones = np.ones(n, np.float32)

    Lrows, Rrows = [], []
    for ap, bp in [(a1, b1), (a1, b2), (a2, b1), (a2, b2), (a3, b1), (a1, b3)]:
        for d in range(DG):
            Lrows.append(ap[:, d])
            Rrows.append(bp[:, d])
    for dd in (d1, d2_, d3):
        Lrows.append(dd)
        Rrows.append(ones)
    for cc in (c1, c2, c3):
        Lrows.append(ones)
        Rrows.append(cc)
    L = np.stack(Lrows).astype(BF16)
    R = np.stack(Rrows).astype(BF16)
    return L, R


def _prepare_inputs(gf0, gf1, node_v_feats, weights, sigmas, n_cores=NCORES):
    """Host-side preprocessing -> per-core in_maps."""
    weights = np.asarray(weights, np.float32)
    sigmas = np.asarray(sigmas, np.float32)
    V = np.asarray(node_v_feats, np.float32)
    n = V.shape[0]
    ni = n // n_cores
    njb = n // 128

    L0, R0full = _build_features(gf0, float(sigmas[0]), float(weights[0]))
    L1, R1full = _build_features(gf1, float(sigmas[1]), float(weights[1]))

    vaug = np.concatenate([V, np.ones((n, 1), np.float32)], axis=1)  # [n, 129]
    va = np.ascontiguousarray(
        vaug.reshape(njb, 128, 129).transpose(1, 0, 2).reshape(128, njb * 129)
    ).astype(np.float16)

    in_maps = []
    for c in range(n_cores):
        rows = slice(c * ni, (c + 1) * ni)
        vr = np.ascontiguousarray(
            V[rows].reshape(ni // 128, 128, DV).transpose(1, 0, 2).reshape(128, ni)
        )
        in_maps.append({
            "L0": np.ascontiguousarray(L0),
            "L1": np.ascontiguousarray(L1),
            "R0": np.ascontiguousarray(R0full[:, rows]),
            "R1": np.ascontiguousarray(R1full[:, rows]),
            "VA": va,
            "VR": vr,
        })
    return in_maps


_NC_CACHE = {}


def _get_nc(n_j=N, n_i=NI):
    key = (n_j, n_i)
    if key not in _NC_CACHE:
        _NC_CACHE[key] = build_nc(n_j, n_i)
    return _NC_CACHE[key]


_EXEC_CACHE = {}


def _get_executor(nc, n_cores):
    """Cached jitted shard_map executor (avoids re-tracing per call)."""
    key = (id(nc), n_cores)
    if key in _EXEC_CACHE:
        return _EXEC_CACHE[key]
    import jax
    from jax.experimental.shard_map import shard_map
    from jax.sharding import Mesh, PartitionSpec
    from concourse.bass2jax import (
        install_neuronx_cc_hook,
        _bass_exec_p,
        partition_id_tensor,
    )

    install_neuronx_cc_hook()

    partition_name = nc.partition_id_tensor.name if nc.partition_id_tensor else None
    in_names, out_names, out_avals = [], [], []
    for alloc in nc.m.functions[0].allocations:
        if not isinstance(alloc, mybir.MemoryLocationSet):
            continue
        name = alloc.memorylocations[0].name
        if alloc.kind == "ExternalInput":
            if name != partition_name:
                in_names.append(name)
        elif alloc.kind == "ExternalOutput":
            out_names.append(name)
            out_avals.append(
                jax.core.ShapedArray(tuple(alloc.tensor_shape), mybir.dt.np(alloc.dtype))
            )
    n_params = len(in_names)
    all_names = list(in_names) + list(out_names)
    if partition_name is not None:
        all_names.append(partition_name)

    def _body(*args):
        operands = list(args)
        if partition_name is not None:
            operands.append(partition_id_tensor())
        outs = _bass_exec_p.bind(
            *operands,
            out_avals=tuple(out_avals),
            in_names=tuple(all_names),
            out_names=tuple(out_names),
            lowering_input_output_aliases=(),
            sim_require_finite=True,
            sim_require_nnan=True,
            nc=nc,
        )
        return tuple(outs)

    devices = jax.devices()[:n_cores]
    mesh = Mesh(np.asarray(devices), ("core",))
    n_outs = len(out_names)
    sharded = jax.jit(
        shard_map(
            _body,
            mesh=mesh,
            in_specs=(PartitionSpec("core"),) * (n_params + n_outs),
            out_specs=(PartitionSpec("core"),) * n_outs,
            check_rep=False,
        ),
        donate_argnums=tuple(range(n_params, n_params + n_outs)),
        keep_unused=True,
    )
    entry = (sharded, in_names, out_names, out_avals)
    _EXEC_CACHE[key] = entry
    return entry


def _run(nc, in_maps, n_cores):
    sharded, in_names, out_names, out_avals = _get_executor(nc, n_cores)
    concat_in = [
        np.concatenate([in_maps[c][name] for c in range(n_cores)], axis=0)
        for name in in_names
    ]
    concat_zeros = [
        np.zeros((n_cores * a.shape[0], *a.shape[1:]), a.dtype) for a in out_avals
    ]
    out_arrs = sharded(*concat_in, *concat_zeros)
    return [
        {
            name: np.asarray(out_arrs[i]).reshape(n_cores, *out_avals[i].shape)[c]
            for i, name in enumerate(out_names)
        }
        for c in range(n_cores)
    ]


def kernel(gf0, gf1, node_v_feats, weights, sigmas):
    in_maps = _prepare_inputs(gf0, gf1, node_v_feats, weights, sigmas)
    nc = _get_nc()
    results = _run(nc, in_maps, NCORES)
    out = np.concatenate([results[c]["out"] for c in range(NCORES)], axis=0)
    return np.ascontiguousarray(out.astype(np.float32))


# revision 15
# speedup vs baseline: 1.0479x; 1.0049x over previous
"""Trainium2 Bass kernel for nn_GAttn_67147518705771.

Computes: score = w0*RBF(gf0, s0) + w1*RBF(gf1, s1)  (N x N)
          attn  = score / (rowsum(score) + 0.01)
          out   = attn @ V + V

Sharding: row-parallel over 8 NeuronCores — core c computes output rows
[c*1024, (c+1)*1024); the key/value side (all N=8192 nodes) is replicated.

Per-core algorithm (all on-chip, no N^2 HBM traffic):
  The exp argument E_m[j,i] = -d2_m[j,i]/(2*sigma_m^2) + ln(w_m) is produced
  directly by ONE bf16 matmul per modality with K=24 augmented feature rows
  (3-way bf16 hi/mid/lo splits of the cross/sq terms give ~fp32 accuracy while
  streaming at bf16 speed, 1 col/cycle). ScalarE exponentiates PSUM->SBUF
  (fp16) in 1536-element ops spanning 3 j-blocks x 2 modalities, DVE adds the
  two modalities, and PE accumulates S @ [V | 1] into persistent PSUM, which
  yields the row sums (ones column) for free. The division by (rowsum + eps)
  commutes with @V, so it is applied to the [128,129] accumulators only, then
  the residual is added and rows stored.

PSUM (8 banks): E tiles [128,1536] (3 banks) double-buffered = 6, U
accumulator [128,258] (1 bank) double-buffered across i-chunks = 2.
start=True clears has_written for the WHOLE bank, so only the first matmul
per accumulation bank sets it.
"""

import numpy as np
import ml_dtypes

import concourse.bass as bass
import concourse.tile as tile
import concourse.mybir as mybir
import concourse.bass_utils as bass_utils

BF16 = ml_dtypes.bfloat16
EPS = 0.01
N = 8192          # total nodes (j / key dim)
DG = 3            # geometric feature dim
DV = 128          # value dim
NCORES = 8
NI = N // NCORES  # query rows per core (1024)
KF = 24           # feature rows per modality
CHUNK = 256       # i-chunk per pass (4 passes per core)
GRP = 3           # j-blocks per E tile / ACT op


def _split_sync_waits(nc, maxw=1):
    """The walrus build in this environment rejects instructions carrying
    more than one sync wait ("Too many sync wait commands"). Hoist excess
    waits onto single-wait InstNoOp carriers inserted just before the owning
    instruction (same engine => same sequencer stream, so ordering-equivalent).
    Returns (n_insts_split, n_carriers)."""
    n_split = n_carriers = 0
    for f in nc.m.functions:
        for bb in f.blocks:
            insts = list(bb.instructions)
            out, changed = [], False
            for inst in insts:
                si = inst.sync_info
                waits = list(si.on_wait) if si and si.on_wait else []
                if len(waits) > maxw:
                    n_split += 1
                    changed = True
                    for w in waits[:-maxw]:
                        nop = mybir.InstNoOp(name=f"waitnop-{n_carriers}", ins=[], outs=[])
                        n_carriers += 1
                        nop.engine = inst.engine
                        nop.sync_info = mybir.SyncInfo(on_wait=[w], on_update=[])
                        out.append(nop)
                    inst.sync_info = mybir.SyncInfo(
                        on_wait=waits[-maxw:], on_update=list(si.on_update or [])
                    )
                out.append(inst)
            if changed:
                bb.instructions = out
    return n_split, n_carriers


def build_nc(n_j=N, n_i=NI):
    """Build the per-core Bass program (SPMD: same program, per-core data)."""
    f32 = mybir.dt.float32
    f16 = mybir.dt.float16
    bf16 = mybir.dt.bfloat16
    njb = n_j // 128
    nchunks = n_i // CHUNK
    # resident input tiles are split into <=8 pieces for fine-grained
    # DMA->compute overlap at startup
    step = (njb + 7) // 8

    nc = bass.Bass("TRN2", target_bir_lowering=False, debug=False)
    L0 = nc.dram_tensor("L0", [KF, n_j], bf16, kind="ExternalInput").ap()
    L1 = nc.dram_tensor("L1", [KF, n_j], bf16, kind="ExternalInput").ap()
    R0 = nc.dram_tensor("R0", [KF, n_i], bf16, kind="ExternalInput").ap()
    R1 = nc.dram_tensor("R1", [KF, n_i], bf16, kind="ExternalInput").ap()
    # V_aug pre-rearranged on host: [128, njb*129] fp16, block jb holds rows
    # jb*128..jb*128+127 of [V | 1].
    VA = nc.dram_tensor("VA", [128, njb * 129], f16, kind="ExternalInput").ap()
    # V residual rows for this core, 128-row-block-major: [128, n_i] f32.
    VR = nc.dram_tensor("VR", [128, n_i], f32, kind="ExternalInput").ap()
    OUT = nc.dram_tensor("out", [n_i, DV], f32, kind="ExternalOutput").ap()

    # First group takes the remainder so (a) the first exp has minimal
    # dependencies and starts early, (b) the kernel tail ends on full groups.
    first = (njb - 1) % GRP + 1
    groups = [list(range(0, first))] + [
        list(range(g, g + GRP)) for g in range(first, njb, GRP)
    ]

    with tile.TileContext(nc) as tc:
        with (
            tc.tile_pool(name="resident", bufs=1) as rpool,
            tc.tile_pool(name="epool", bufs=2, space="PSUM") as epool,
            tc.tile_pool(name="upool", bufs=2, space="PSUM") as upool,
            tc.tile_pool(name="spool", bufs=3) as spool,
            tc.tile_pool(name="sumpool", bufs=3) as sumpool,
            tc.tile_pool(name="opool", bufs=4) as opool,
            tc.tile_pool(name="scalars", bufs=4) as scpool,
        ):
            # Spread the first group's operands across independent DMA-issue
            # rails (each rail serializes at ~0.6-1us per dma_start): the
            # critical chain for the first E matmuls is r0+l0p0 (sync rail),
            # r1 (gpsimd) and l1p0 (tensor rail); va0 follows on gpsimd.
            # Later pieces are interleaved on sync in compute order.
            r0_sb = rpool.tile([KF, n_i], bf16)
            nc.sync.dma_start(r0_sb[:], R0[:])
            r1_sb = rpool.tile([KF, n_i], bf16)
            nc.gpsimd.dma_start(r1_sb[:], R1[:])

            l_tiles = [[], []]
            va_tiles = []
            for p in range(0, njb, step):
                p2 = min(njb, p + step)
                for m, Lap in ((0, L0), (1, L1)):
                    eng = nc.sync if p > 0 else (nc.sync if m == 0 else nc.scalar)
                    t = rpool.tile([KF, (p2 - p) * 128], bf16, name=f"l{m}_{p}")
                    eng.dma_start(t[:], Lap[:, p * 128:p2 * 128])
                    l_tiles[m].append(t)
                eng = nc.gpsimd if p == 0 else nc.sync
                t = rpool.tile([128, (p2 - p) * 129], f16, name=f"va_{p}")
                eng.dma_start(t[:], VA[:, p * 129:p2 * 129])
                va_tiles.append(t)

            vr_sb = rpool.tile([128, n_i], f32)
            nc.sync.dma_start(vr_sb[:], VR[:])

            # Dummy exp (after the ACT-rail DMA issue) pre-loads the ACT
            # exp-table while the input DMAs stream in.
            dummy = scpool.tile([128, 1], f32, tag="dummy")
            nc.vector.memset(dummy[:], 0.0)
            dummy2 = scpool.tile([128, 1], f32, tag="dummy2")
            nc.scalar.activation(dummy2[:], dummy[:], mybir.ActivationFunctionType.Exp)

            def lsl(m, jb):  # lhsT feature slice [KF, 128] for modality m
                t = l_tiles[m][jb // step]
                o = (jb % step) * 128
                return t[:, o:o + 128]

            def vasl(jb):  # V_aug block [128, 129]
                t = va_tiles[jb // step]
                o = (jb % step) * 129
                return t[:, o:o + 129]

            # Chunks are processed in interleaved PAIRS: while chunk c0's exp
            # runs on ScalarE, the PE computes chunk c1's E matmuls, so the
            # PE program order never stalls on the last exp of a chunk except
            # at pair boundaries.
            assert nchunks % 2 == 0
            for cpair in range(nchunks // 2):
                chunks = (2 * cpair, 2 * cpair + 1)
                # Per-chunk U accumulator: 2 subblocks x 129 cols in ONE bank.
                u_t = [upool.tile([128, 2 * 129], f32, tag="u", name=f"u_{c}")
                       for c in chunks]

                for jbs in groups:
                    w = len(jbs) * 512
                    for ci, chunk in enumerate(chunks):
                        c0 = chunk * CHUNK
                        u = u_t[ci]
                        e = epool.tile([128, GRP * 512], f32, tag="e",
                                       name=f"e_{chunk}_{jbs[0]}")
                        for t, jb in enumerate(jbs):
                            nc.tensor.matmul(
                                e[:, t * 512:t * 512 + 256],
                                lhsT=lsl(0, jb),
                                rhs=r0_sb[:, c0:c0 + CHUNK],
                                start=True, stop=True,
                            )
                            nc.tensor.matmul(
                                e[:, t * 512 + 256:(t + 1) * 512],
                                lhsT=lsl(1, jb),
                                rhs=r1_sb[:, c0:c0 + CHUNK],
                                start=True, stop=True,
                            )
                        s = spool.tile([128, GRP * 512], f16, tag="s",
                                       name=f"s_{chunk}_{jbs[0]}")
                        nc.scalar.activation(
                            s[:, 0:w], e[:, 0:w], mybir.ActivationFunctionType.Exp
                        )
                        ss = sumpool.tile([128, GRP * 256], f16, tag="ss",
                                          name=f"ss_{chunk}_{jbs[0]}")
                        for t, jb in enumerate(jbs):
                            nc.vector.tensor_add(
                                ss[:, t * 256:(t + 1) * 256],
                                s[:, t * 512:t * 512 + 256],
                                s[:, t * 512 + 256:(t + 1) * 512],
                            )
                        for t, jb in enumerate(jbs):
                            for isub in range(2):
                                # start=True clears has_written for the WHOLE
                                # bank: only the first matmul touching the
                                # bank in this chunk sets it; isub 1's first
                                # write lands on cleared bits (= overwrite)
                                # and later j-blocks accumulate.
                                nc.tensor.matmul(
                                    u[:, isub * 129:(isub + 1) * 129],
                                    lhsT=ss[:, t * 256 + isub * 128:t * 256 + (isub + 1) * 128],
                                    rhs=vasl(jb),
                                    start=(jb == 0 and isub == 0),
                                    stop=(jb == njb - 1),
                                    skip_group_check=True,
                                )

                for ci, chunk in enumerate(chunks):
                    for isub in range(2):
                        g = chunk * 2 + isub
                        ut = u_t[ci][:, isub * 129:(isub + 1) * 129]
                        rt = scpool.tile([128, 1], f32, tag="rt", name=f"rt_{g}")
                        nc.vector.tensor_scalar_add(rt[:], ut[:, 128:129], EPS)
                        ri = scpool.tile([128, 1], f32, tag="ri", name=f"ri_{g}")
                        nc.vector.reciprocal(ri[:], rt[:])
                        ot = opool.tile([128, DV], f32, tag="ot", name=f"ot_{g}")
                        nc.vector.tensor_scalar_mul(ot[:], ut[:, 0:DV], ri[:])
                        nc.vector.tensor_add(ot[:], ot[:], vr_sb[:, g * 128:(g + 1) * 128])
                        out_eng = nc.sync if isub == 0 else nc.gpsimd
                        out_eng.dma_start(OUT[g * 128:(g + 1) * 128, :], ot[:])

    _split_sync_waits(nc)
    return nc


def _split3(v):
    v1 = v.astype(BF16).astype(np.float32)
    v2 = (v - v1).astype(BF16).astype(np.float32)
    v3 = (v - v1 - v2).astype(BF16).astype(np.float32)
    return v1, v2, v3


def _build_features(gf, sigma, w):
    """L [KF, N] (j-side) and R [KF, N] (i-side) bf16 feature rows such that
    (L.T @ R)[j, i] = -d2[j,i]/(2 sigma^2) + ln(w) to ~1e-5."""
    gf = np.asarray(gf, dtype=np.float32)
    n = gf.shape[0]
    g = np.float32(1.0 / (2.0 * sigma * sigma))
    sq = (gf * gf).sum(axis=1)
    a = 2.0 * g * gf            # j-side cross
    b = gf                      # i-side cross
    dterm = -g * sq             # j-side
    c = -g * sq + np.float32(np.log(w))  # i-side

    a1, a2, a3 = _split3(a)
    b1, b2, b3 = _split3(b)
    d1, d2_, d3 = _split3(dterm)
    c1, c2, c3 = _split3(c)
    ones = np.ones(n, np.float32)

    Lrows, Rrows = [], []
    for ap, bp in [(a1, b1), (a1, b2), (a2, b1), (a2, b2), (a3, b1), (a1, b3)]:
        for d in range(DG):
            Lrows.append(ap[:, d])
            Rrows.append(bp[:, d])
    for dd in (d1, d2_, d3):
        Lrows.append(dd)
        Rrows.append(ones)
    for cc in (c1, c2, c3):
        Lrows.append(ones)
        Rrows.append(cc)
    L = np.stack(Lrows).astype(BF16)
    R = np.stack(Rrows).astype(BF16)
    return L, R


def _prepare_inputs(gf0, gf1, node_v_feats, weights, sigmas, n_cores=NCORES):
    """Host-side preprocessing -> per-core in_maps."""
    weights = np.asarray(weights, np.float32)
    sigmas = np.asarray(sigmas, np.float32)
    V = np.asarray(node_v_feats, np.float32)
    n = V.shape[0]
    ni = n // n_cores
    njb = n // 128

    L0, R0full = _build_features(gf0, float(sigmas[0]), float(weights[0]))
    L1, R1full = _build_features(gf1, float(sigmas[1]), float(weights[1]))

    vaug = np.concatenate([V, np.ones((n, 1), np.float32)], axis=1)  # [n, 129]
    va = np.ascontiguousarray(
        vaug.reshape(njb, 128, 129).transpose(1, 0, 2).reshape(128, njb * 129)
    ).astype(np.float16)

    in_maps = []
    for c in range(n_cores):
        rows = slice(c * ni, (c + 1) * ni)
        vr = np.ascontiguousarray(
            V[rows].reshape(ni // 128, 128, DV).transpose(1, 0, 2).reshape(128, ni)
        )
        in_maps.append({
            "L0": np.ascontiguousarray(L0),
            "L1": np.ascontiguousarray(L1),
            "R0": np.ascontiguousarray(R0full[:, rows]),
            "R1": np.ascontiguousarray(R1full[:, rows]),
            "VA": va,
            "VR": vr,
        })
    return in_maps


_NC_CACHE = {}


def _get_nc(n_j=N, n_i=NI):
    key = (n_j, n_i)
    if key not in _NC_CACHE:
        _NC_CACHE[key] = build_nc(n_j, n_i)
    return _NC_CACHE[key]


_EXEC_CACHE = {}


def _get_executor(nc, n_cores):
    """Cached jitted shard_map executor (avoids re-tracing per call)."""
    key = (id(nc), n_cores)
    if key in _EXEC_CACHE:
        return _EXEC_CACHE[key]
    import jax
    from jax.experimental.shard_map import shard_map
    from jax.sharding import Mesh, PartitionSpec
    from concourse.bass2jax import (
        install_neuronx_cc_hook,
        _bass_exec_p,
        partition_id_tensor,
    )

    install_neuronx_cc_hook()

    partition_name = nc.partition_id_tensor.name if nc.partition_id_tensor else None
    in_names, out_names, out_avals = [], [], []
    for alloc in nc.m.functions[0].allocations:
        if not isinstance(alloc, mybir.MemoryLocationSet):
            continue
        name = alloc.memorylocations[0].name
        if alloc.kind == "ExternalInput":
            if name != partition_name:
                in_names.append(name)
        elif alloc.kind == "ExternalOutput":
            out_names.append(name)
            out_avals.append(
                jax.core.ShapedArray(tuple(alloc.tensor_shape), mybir.dt.np(alloc.dtype))
            )
    n_params = len(in_names)
    all_names = list(in_names) + list(out_names)
    if partition_name is not None:
        all_names.append(partition_name)

    def _body(*args):
        operands = list(args)
        if partition_name is not None:
            operands.append(partition_id_tensor())
        outs = _bass_exec_p.bind(
            *operands,
            out_avals=tuple(out_avals),
            in_names=tuple(all_names),
            out_names=tuple(out_names),
            lowering_input_output_aliases=(),
            sim_require_finite=True,
            sim_require_nnan=True,
            nc=nc,
        )
        return tuple(outs)

    devices = jax.devices()[:n_cores]
    mesh = Mesh(np.asarray(devices), ("core",))
    n_outs = len(out_names)
    sharded = jax.jit(
        shard_map(
            _body,
            mesh=mesh,
            in_specs=(PartitionSpec("core"),) * (n_params + n_outs),
            out_specs=(PartitionSpec("core"),) * n_outs,
            check_rep=False,
        ),
        donate_argnums=tuple(range(n_params, n_params + n_outs)),
        keep_unused=True,
    )
    entry = (sharded, in_names, out_names, out_avals)
    _EXEC_CACHE[key] = entry
    return entry


def _run(nc, in_maps, n_cores):
    sharded, in_names, out_names, out_avals = _get_executor(nc, n_cores)
    concat_in = [
        np.concatenate([in_maps[c][name] for c in range(n_cores)], axis=0)
        for name in in_names
    ]
    concat_zeros = [
        np.zeros((n_cores * a.shape[0], *a.shape[1:]), a.dtype) for a in out_avals
    ]
    out_arrs = sharded(*concat_in, *concat_zeros)
    return [
        {
            name: np.asarray(out_arrs[i]).reshape(n_cores, *out_avals[i].shape)[c]
            for i, name in enumerate(out_names)
        }
        for c in range(n_cores)
    ]


def kernel(gf0, gf1, node_v_feats, weights, sigmas):
    in_maps = _prepare_inputs(gf0, gf1, node_v_feats, weights, sigmas)
    nc = _get_nc()
    results = _run(nc, in_maps, NCORES)
    out = np.concatenate([results[c]["out"] for c in range(NCORES)], axis=0)
    return np.ascontiguousarray(out.astype(np.float32))
